# revision 23
# baseline (speedup 1.0000x reference)
"""Trainium2 Bass kernel for nn_CustomMultiHeadAttention_57131654971614.

Computes multi-head attention equivalent to:
    Q = xq @ w_q ; K = xk @ w_k ; V = xv @ w_v   (per head)
    S = Q K^T / sqrt(dk) ; P = softmax(S)        (mask is all-ones)
    out = sum_h (P V) @ w_o

Sharding: 8 cores = 2 batches x 4 head-groups (4 heads each).
Each core computes its batch's partial output summed over its 4 heads;
host sums the 4 partials per batch.

v2 design notes (vs the transpose-heavy v1, ~9x faster on HW):
  - host feeds x^T (so no on-device x transposes) and fp16 operands
    everywhere (PSUM accumulation stays fp32; rel err ~8e-4)
  - V is computed directly in [t, dv] layout (no V transposes); softmax
    denominators come from memset ones-columns interleaved into the V
    tile so the PV matmul also produces the rowsums (65-wide stationary)
  - all matmul outputs are 512 fp32 wide (one PSUM bank - wider outputs
    fail walrus codegen: s3d3_mm_num_elements); DMA staging is 1024-wide
  - normalization uses the v1 transpose scheme (transpose / reciprocal /
    tensor_scalar_mul / transpose back); exotic constructs (1-partition
    stationary broadcast matmul, output-base-64 shift matmul) crashed
    the device with NRT_EXEC_UNIT_UNRECOVERABLE despite passing CoreSim
"""

import sys

for _p in ("/opt/trn_rl_repo", "/root/.axon_site/_ro/trn_rl_repo"):
    if _p not in sys.path:
        sys.path.insert(0, _p)

from contextlib import ExitStack

import numpy as np

import concourse.bass as bass  # noqa: F401  (import keeps bass registered)
import concourse.mybir as mybir
import concourse.tile as tile
from concourse import bacc
from concourse.bass_utils import run_bass_kernel_spmd
from concourse.masks import make_identity

B, T, E = 2, 2048, 1024
H, DK, DV = 16, 64, 64
NCORES = 8
HPC = 4          # heads per core
NPAIR = 2        # head pairs per core
SCALE = 1.0 / 8.0  # 1/sqrt(DK)

F32 = mybir.dt.float32
F16 = mybir.dt.float16
EXP = mybir.ActivationFunctionType.Exp
MUL = mybir.AluOpType.mult

EC = E // 128    # 8 e-chunks of 128
KC = T // 128    # 16 key chunks of 128
QB = 512         # q-block width (max psum-bank-legal matmul width)
NQB = T // QB    # 4 q-blocks
TCH = T // 128   # 16 t-chunks
XB = 1024        # x staging block width (DMA granularity)


def build_nc(nrep: int = 1, **_kw):
    """Build the SPMD Bass program (same program on all 8 cores)."""
    nc = bacc.Bacc("TRN2", target_bir_lowering=False, debug=False,
                   num_devices=NCORES)
    xqT_d = nc.dram_tensor("xqT", [128, EC, T], F16, kind="ExternalInput")
    xkT_d = nc.dram_tensor("xkT", [128, EC, T], F16, kind="ExternalInput")
    xvT_d = nc.dram_tensor("xvT", [128, EC, T], F16, kind="ExternalInput")
    # [128, EC, pair, 2*DK]: per pair, two heads' w_q/w_k cat on last dim
    wq_d = nc.dram_tensor("wq", [128, EC, NPAIR, 128], F16,
                          kind="ExternalInput")
    wk_d = nc.dram_tensor("wk", [128, EC, NPAIR, 128], F16,
                          kind="ExternalInput")
    # [128, EC, HPC, DV]
    wv_d = nc.dram_tensor("wv", [128, EC, HPC, DV], F16, kind="ExternalInput")
    # [128, pair, E]: partitions = pair-packed dv
    wo_d = nc.dram_tensor("wo", [128, NPAIR, E], F16, kind="ExternalInput")
    out_d = nc.dram_tensor("out", [T, E], F16, kind="ExternalOutput")

    with tile.TileContext(nc) as tc:
        _emit(nc, tc, nrep, (xqT_d, xkT_d, xvT_d), (wq_d, wk_d), wv_d, wo_d,
              out_d)
    nc.compile()
    return nc


def _emit(nc, tc, nrep, xT_ds, wqk_ds, wv_d, wo_d, out_d):
    xqT_d, xkT_d, xvT_d = xT_ds
    ctx = ExitStack()
    with ctx:
        const = ctx.enter_context(tc.tile_pool(name="const", bufs=1))
        xvp = ctx.enter_context(tc.tile_pool(name="xvp", bufs=1))
        xsp = ctx.enter_context(tc.tile_pool(name="xsp", bufs=2))
        qkp = ctx.enter_context(tc.tile_pool(name="qkp", bufs=2))
        vap = ctx.enter_context(tc.tile_pool(name="vap", bufs=2))
        ptp = ctx.enter_context(tc.tile_pool(name="ptp", bufs=3))
        ontp = ctx.enter_context(tc.tile_pool(name="ontp", bufs=2))
        smll = ctx.enter_context(tc.tile_pool(name="smll", bufs=3))
        onrmp = ctx.enter_context(tc.tile_pool(name="onrmp", bufs=8))
        outp = ctx.enter_context(tc.tile_pool(name="outp", bufs=3))
        # PSUM (8 banks): "big" ring 3 + "po" 2 + "v" 1 + "tr" 2
        psB = ctx.enter_context(tc.tile_pool(name="psB", bufs=3, space="PSUM"))
        psP = ctx.enter_context(tc.tile_pool(name="psP", bufs=2, space="PSUM"))
        psV = ctx.enter_context(tc.tile_pool(name="psV", bufs=1, space="PSUM"))
        psT = ctx.enter_context(tc.tile_pool(name="psT", bufs=2, space="PSUM"))

        # ---- constants + weights (outside the timing loop) ----
        ident = const.tile([128, 128], F16, tag="ident")
        make_identity(nc, ident)

        wq_r = const.tile([128, EC, NPAIR, 128], F16, tag="wq")
        nc.sync.dma_start(out=wq_r, in_=wqk_ds[0][:, :, :, :])
        wk_r = const.tile([128, EC, NPAIR, 128], F16, tag="wk")
        nc.sync.dma_start(out=wk_r, in_=wqk_ds[1][:, :, :, :])
        wv_r = const.tile([128, EC, HPC * DV], F16, tag="wv")
        nc.sync.dma_start(out=wv_r,
                          in_=wv_d.rearrange("p c h d -> p c (h d)"))
        wo_r = const.tile([128, NPAIR, E], F16, tag="wo")
        nc.sync.dma_start(out=wo_r, in_=wo_d[:, :, :])

        def body(_iv=None):
            # ---- stage 1a: V in [t, (h, dv)] layout + ones columns ----
            va = vap.tile([128, KC, HPC, DV + 1], F16, tag="va")
            nc.vector.memset(va[:, :, :, DV], 1.0)
            xv_sb = xvp.tile([128, EC, T], F16, tag="xv")
            nc.sync.dma_start(out=xv_sb, in_=xvT_d[:, :, :])
            for tc_i in range(TCH):
                ts = slice(tc_i * 128, (tc_i + 1) * 128)
                vp = psV.tile([128, HPC * DV], F32, tag="v")
                for c in range(EC):
                    nc.tensor.matmul(
                        vp, xv_sb[:, c, ts], wv_r[:, c, :],
                        start=(c == 0), stop=(c == EC - 1))
                nc.vector.tensor_copy(out=va[:, tc_i, :, :DV], in_=vp)

            # ---- stage 1b: K^T then Q^T (pair-packed on partitions) ----
            kt = qkp.tile([128, NPAIR, T], F16, tag="kt")
            qt = qkp.tile([128, NPAIR, T], F16, tag="qt")
            for dst, src_d, w_r in ((kt, xkT_d, wk_r), (qt, xqT_d, wq_r)):
                for tb in range(T // XB):
                    bs = slice(tb * XB, (tb + 1) * XB)
                    x_sb = xsp.tile([128, EC, XB], F16, tag="xs")
                    nc.sync.dma_start(out=x_sb, in_=src_d[:, :, bs])
                    for r in range(NPAIR):
                        for sb in range(XB // QB):
                            ps = psB.tile([128, QB], F32, tag="big")
                            for c in range(EC):
                                nc.tensor.matmul(
                                    ps, w_r[:, c, r, :],
                                    x_sb[:, c, sb * QB:(sb + 1) * QB],
                                    start=(c == 0), stop=(c == EC - 1))
                            nc.vector.tensor_copy(
                                out=dst[:, r,
                                        tb * XB + sb * QB:
                                        tb * XB + (sb + 1) * QB],
                                in_=ps)

            # ---- stage 2: attention per (pair, head, q-block) ----
            ont = ontp.tile([128, NPAIR, T], F16, tag="ont")
            for r in range(NPAIR):
                for qb in range(NQB):
                    qs = slice(qb * QB, (qb + 1) * QB)
                    # normalized O for both heads of the pair, [q, 2*DV]
                    onrm = [onrmp.tile([128, 128], F16, tag="onrm",
                                       name=f"onrm_{r}_{qb}_{j}")
                            for j in range(QB // 128)]
                    for i in range(2):
                        h = 2 * r + i
                        hs = slice(i * 64, (i + 1) * 64)
                        po = psP.tile([128, QB], F32, tag="po")
                        for kc in range(KC):
                            sp = psB.tile([128, QB], F32, tag="big")
                            nc.tensor.matmul(
                                sp, kt[hs, r, kc * 128:(kc + 1) * 128],
                                qt[hs, r, qs], start=True, stop=True)
                            pt = ptp.tile([128, QB], F16, tag="pt")
                            nc.scalar.activation(
                                out=pt, in_=sp, func=EXP, scale=SCALE)
                            nc.tensor.matmul(
                                po[:DV + 1, :], va[:, kc, h, :], pt,
                                start=(kc == 0), stop=(kc == KC - 1))
                        # normalize via transpose: O[q,:] /= rowsum[q]
                        oa = smll.tile([DV + 1, QB], F16, tag="oa")
                        nc.vector.tensor_copy(out=oa, in_=po[:DV + 1, :])
                        for qc in range(QB // 128):
                            ps1 = psT.tile([128, 128], F16, tag="tr")
                            nc.tensor.transpose(
                                ps1[:, :DV + 1],
                                oa[:, qc * 128:(qc + 1) * 128],
                                ident[:DV + 1, :DV + 1])
                            oasb = smll.tile([128, DV + 1], F16, tag="oasb")
                            nc.vector.tensor_copy(out=oasb,
                                                  in_=ps1[:, :DV + 1])
                            rec = smll.tile([128, 1], F32, tag="rec")
                            nc.vector.reciprocal(rec, oasb[:, DV:DV + 1])
                            nc.vector.tensor_scalar_mul(
                                onrm[qc][:, i * 64:(i + 1) * 64],
                                oasb[:, :DV], rec)
                    # transpose [q, 2*DV] -> pair-packed [2*DV, q]
                    for qc in range(QB // 128):
                        ps2 = psT.tile([128, 128], F16, tag="tr")
                        nc.tensor.transpose(ps2, onrm[qc], ident)
                        nc.vector.tensor_copy(
                            out=ont[:, r,
                                    qb * QB + qc * 128:
                                    qb * QB + (qc + 1) * 128],
                            in_=ps2)

            # ---- stage 3: output projection, summed over heads ----
            for qc in range(TCH):
                cs = slice(qc * 128, (qc + 1) * 128)
                ot = outp.tile([128, E], F16, tag="ot")
                for eb in range(E // QB):
                    es = slice(eb * QB, (eb + 1) * QB)
                    pf = psB.tile([128, QB], F32, tag="big")
                    for r in range(NPAIR):
                        nc.tensor.matmul(
                            pf, ont[:, r, cs], wo_r[:, r, es],
                            start=(r == 0), stop=(r == NPAIR - 1))
                    nc.vector.tensor_copy(out=ot[:, es], in_=pf)
                nc.sync.dma_start(out=out_d[cs, :], in_=ot)

        if nrep == 1:
            body()
        else:
            with tc.For_i(0, nrep, 1):
                body()


def pack_inputs(x_query, x_key, x_value, w_q, w_k, w_v, w_o):
    """Split full inputs into 8 per-core input maps (fp16, transposed)."""
    x_query = np.asarray(x_query, dtype=np.float32)
    x_key = np.asarray(x_key, dtype=np.float32)
    x_value = np.asarray(x_value, dtype=np.float32)
    w_q = np.asarray(w_q, dtype=np.float32)
    w_k = np.asarray(w_k, dtype=np.float32)
    w_v = np.asarray(w_v, dtype=np.float32)
    w_o = np.asarray(w_o, dtype=np.float32)

    def xt_pack(xb):
        # [T, E] -> [128, EC, T] with e = c*128 + p
        return np.ascontiguousarray(
            xb.T.reshape(EC, 128, T).transpose(1, 0, 2).astype(np.float16))

    xq_p = [xt_pack(x_query[b]) for b in range(B)]
    xk_p = [xt_pack(x_key[b]) for b in range(B)]
    xv_p = [xt_pack(x_value[b]) for b in range(B)]

    in_maps = []
    for c in range(NCORES):
        b, g = divmod(c, 4)
        h0 = HPC * g
        # [E, pair, 128] -> [128, EC, pair, 128]
        wq_c = np.stack([np.concatenate([w_q[h0 + 2 * r], w_q[h0 + 2 * r + 1]],
                                        axis=1) for r in range(NPAIR)], axis=1)
        wk_c = np.stack([np.concatenate([w_k[h0 + 2 * r], w_k[h0 + 2 * r + 1]],
                                        axis=1) for r in range(NPAIR)], axis=1)
        wq_c = wq_c.reshape(EC, 128, NPAIR, 128).transpose(1, 0, 2, 3)
        wk_c = wk_c.reshape(EC, 128, NPAIR, 128).transpose(1, 0, 2, 3)
        # [E, HPC, DV] -> [128, EC, HPC, DV]
        wv_c = np.stack([w_v[h0 + h] for h in range(HPC)], axis=1)
        wv_c = wv_c.reshape(EC, 128, HPC, DV).transpose(1, 0, 2, 3)
        # [pair, 2*DV, E] -> [128, pair, E]
        wo_c = np.stack([np.concatenate([w_o[h0 + 2 * r], w_o[h0 + 2 * r + 1]],
                                        axis=0) for r in range(NPAIR)], axis=0)
        wo_c = wo_c.transpose(1, 0, 2)
        in_maps.append({
            "xqT": xq_p[b],
            "xkT": xk_p[b],
            "xvT": xv_p[b],
            "wq": np.ascontiguousarray(wq_c.astype(np.float16)),
            "wk": np.ascontiguousarray(wk_c.astype(np.float16)),
            "wv": np.ascontiguousarray(wv_c.astype(np.float16)),
            "wo": np.ascontiguousarray(wo_c.astype(np.float16)),
        })
    return in_maps


def unpack_outputs(results):
    """Sum the 4 head-group partials per batch."""
    out = np.zeros((B, T, E), dtype=np.float32)
    for c in range(NCORES):
        b = c // 4
        out[b] += results[c]["out"].astype(np.float32)
    return out


_NC_CACHE = {}


def kernel(x_query, x_key, x_value, mask, w_q, w_k, w_v, w_o):
    key = "main"
    if key not in _NC_CACHE:
        _NC_CACHE[key] = build_nc(nrep=1)
    nc = _NC_CACHE[key]
    in_maps = pack_inputs(x_query, x_key, x_value, w_q, w_k, w_v, w_o)
    res = run_bass_kernel_spmd(nc, in_maps, list(range(NCORES)))
    return unpack_outputs(res.results)
